# revision 3
# baseline (speedup 1.0000x reference)
"""Multi-head attention (B=2, S=2048, D=1024, H=16) on 8 NeuronCores.

Sharding: core = (batch b, head-group hg) with b in {0,1}, hg in {0..3}.
Each core computes 4 heads (256 of the 1024 hidden dims) for one batch
element and produces a partial output [S, D]; the host sums the 4
head-group partials per batch and adds the output bias.

Per-core dataflow (all matmuls bf16 inputs, fp32 PSUM accumulation):
  Q^T = Wq_c^T @ x^T          [256, S]   (c on partitions)
  K^T = Wk_c^T @ x^T          [256, S]
  V   = x @ Wv_c  (+ bias)    [S, 256]   (s on partitions)
  scores^T[k,q] = K^T(h)^T-slices @ Q^T(h)   (per head, k on partitions)
  attn^T = exp(scores^T / 8)  (no max subtraction needed: |scores/8| < ~2)
  ctx_aug^T = [ones | V_h]^T @ attn^T  -> row 0 = softmax denominator,
                                          rows 1..64 = unnormalized ctx^T
  ctx^T = ctx_aug^T * broadcast(1/denom)
  out_partial = sum_h ctx_h^T^T-slices @ Wo_aug_h   (K=65, zero row kills
                                                     the 1.0 row)
"""

from contextlib import ExitStack

import ml_dtypes
import numpy as np

import concourse.bass as bass
import concourse.mybir as mybir
import concourse.tile as tile
from concourse import bacc
from concourse.bass import ts
from concourse import bass_utils

S = 2048
D = 1024
H = 16
HD = 64
HPC = 4          # heads per core
C = HPC * HD     # 256 hidden dims per core
N_CORES = 8

BF16 = mybir.dt.bfloat16
F32 = mybir.dt.float32
NP_BF16 = ml_dtypes.bfloat16

_CACHE = {}


def _build_nc():
    nc = bacc.Bacc(
        "TRN2", target_bir_lowering=False, debug=False, num_devices=N_CORES
    )

    xT = nc.dram_tensor("xT", [D, S], BF16, kind="ExternalInput").ap()
    wq = nc.dram_tensor("wq", [D, C], BF16, kind="ExternalInput").ap()
    wk = nc.dram_tensor("wk", [D, C], BF16, kind="ExternalInput").ap()
    wv = nc.dram_tensor("wv", [D, C], BF16, kind="ExternalInput").ap()
    wo = nc.dram_tensor("wo", [HPC, HD + 1, D], BF16, kind="ExternalInput").ap()
    bq = nc.dram_tensor("bq", [2, 128, 1], F32, kind="ExternalInput").ap()
    bk = nc.dram_tensor("bk", [2, 128, 1], F32, kind="ExternalInput").ap()
    bv = nc.dram_tensor("bv", [1, C], BF16, kind="ExternalInput").ap()
    out = nc.dram_tensor("out", [S, D], F32, kind="ExternalOutput").ap()

    with tile.TileContext(nc) as tc, ExitStack() as ctx:
        ep = ctx.enter_context

        xt_pool = ep(tc.tile_pool(name="xt", bufs=8))
        w_pool = ep(tc.tile_pool(name="w", bufs=24))
        wo_pool = ep(tc.tile_pool(name="wo", bufs=4))
        small_pool = ep(tc.tile_pool(name="small", bufs=6))
        qk_pool = ep(tc.tile_pool(name="qk", bufs=4))
        vaug_pool = ep(tc.tile_pool(name="vaug", bufs=16))
        ctx_pool = ep(tc.tile_pool(name="ctxp", bufs=16))
        attn_pool = ep(tc.tile_pool(name="attn", bufs=6))
        recip_pool = ep(tc.tile_pool(name="recip", bufs=4))
        bcast_pool = ep(tc.tile_pool(name="bcast", bufs=4))
        outsb_pool = ep(tc.tile_pool(name="outsb", bufs=4))
        mm_ps = ep(tc.tile_pool(name="mmps", bufs=2, space="PSUM"))
        sc_ps = ep(tc.tile_pool(name="scps", bufs=2, space="PSUM"))
        cx_ps = ep(tc.tile_pool(name="cxps", bufs=2, space="PSUM"))

        # ---- load everything ----
        xt = []
        for k in range(8):
            t = xt_pool.tile([128, S], BF16, tag="xt")
            nc.sync.dma_start(t[:], xT[ts(k, 128), :])
            xt.append(t)

        wsb = {}
        for name, src in (("wq", wq), ("wk", wk), ("wv", wv)):
            tiles = []
            for k in range(8):
                t = w_pool.tile([128, C], BF16, tag="w")
                nc.sync.dma_start(t[:], src[ts(k, 128), :])
                tiles.append(t)
            wsb[name] = tiles

        wo_sb = []
        for h in range(HPC):
            t = wo_pool.tile([HD + 1, D], BF16, tag="wo")
            nc.sync.dma_start(t[:], wo[h])
            wo_sb.append(t)

        bq_sb, bk_sb = [], []
        for m in range(2):
            t = small_pool.tile([128, 1], F32, tag="bqk")
            nc.sync.dma_start(t[:], bq[m])
            bq_sb.append(t)
        for m in range(2):
            t = small_pool.tile([128, 1], F32, tag="bqk")
            nc.sync.dma_start(t[:], bk[m])
            bk_sb.append(t)

        bv_row = small_pool.tile([1, C], BF16, tag="bv")
        nc.sync.dma_start(bv_row[:], bv[:])
        ones_row = small_pool.tile([1, 128], BF16, tag="ones")
        nc.vector.memset(ones_row[:], 1.0)

        # ---- K / Q projections: dst^T[c, q] (c on partitions) ----
        kt, qt = [], []
        for dst_list, w_tiles, b_tiles in (
            (kt, wsb["wk"], bk_sb),
            (qt, wsb["wq"], bq_sb),
        ):
            for m in range(2):
                dst = qk_pool.tile([128, S], BF16, tag="qk")
                dst_list.append(dst)
                for n in range(4):
                    ps = mm_ps.tile([128, 512], F32, tag="mm")
                    for k in range(8):
                        nc.tensor.matmul(
                            ps[:],
                            lhsT=w_tiles[k][:, ts(m, 128)],
                            rhs=xt[k][:, ts(n, 512)],
                            start=(k == 0),
                            stop=(k == 7),
                        )
                    nc.vector.tensor_scalar(
                        dst[:, ts(n, 512)],
                        ps[:],
                        b_tiles[m][:],
                        None,
                        mybir.AluOpType.add,
                    )

        # ---- V projection into augmented layout [ones | V_h] per head ----
        vaug = []
        for s in range(16):
            ps = mm_ps.tile([128, C], F32, tag="mm")
            for k in range(8):
                nc.tensor.matmul(
                    ps[:],
                    lhsT=xt[k][:, ts(s, 128)],
                    rhs=wsb["wv"][k][:],
                    start=(k == 0),
                    stop=False,
                )
            nc.tensor.matmul(
                ps[:],
                lhsT=ones_row[:],
                rhs=bv_row[:],
                start=False,
                stop=True,
            )
            vt = vaug_pool.tile([128, HPC * (HD + 1)], BF16, tag="vaug")
            vt3 = vt[:].rearrange("p (h x) -> p h x", x=HD + 1)
            nc.vector.memset(vt3[:, :, 0:1], 1.0)
            nc.vector.tensor_copy(
                vt3[:, :, 1 : HD + 1],
                ps[:].rearrange("p (h d) -> p h d", d=HD),
            )
            vaug.append(vt)

        # ---- attention + output projection, per q-chunk ----
        ctx_tiles = {}
        for n in range(4):
            for p in range(2):
                cxs = [
                    cx_ps.tile(
                        [HD + 1, 512], F32, tag="cx", name=f"cx_{n}_{p}_{hh}"
                    )
                    for hh in range(2)
                ]
                for j in range(8):
                    scs = [
                        sc_ps.tile(
                            [128, 1024], F32, tag="sc", name=f"sc_{n}_{p}_{j}_{hh}"
                        )
                        for hh in range(2)
                    ]
                    for tt in range(2):
                        t = 2 * j + tt
                        for hh in range(2):
                            nc.tensor.matmul(
                                scs[hh][:, ts(tt, 512)],
                                lhsT=kt[p][64 * hh : 64 * hh + 64, ts(t, 128)],
                                rhs=qt[p][64 * hh : 64 * hh + 64, ts(n, 512)],
                                start=True,
                                stop=True,
                                tile_position=(64 * hh, 0),
                            )
                    ats = []
                    for hh in range(2):
                        at = attn_pool.tile([128, 1024], BF16, tag="at")
                        nc.scalar.activation(
                            at[:],
                            scs[hh][:],
                            mybir.ActivationFunctionType.Exp,
                            scale=0.125,
                        )
                        ats.append(at)
                    for tt in range(2):
                        t = 2 * j + tt
                        for hh in range(2):
                            h_local = 2 * p + hh
                            nc.tensor.matmul(
                                cxs[hh][:],
                                lhsT=vaug[t][
                                    :, 65 * h_local : 65 * h_local + 65
                                ],
                                rhs=ats[hh][:, ts(tt, 512)],
                                start=(t == 0),
                                stop=(t == 15),
                            )
                for hh in range(2):
                    h_local = 2 * p + hh
                    rc = recip_pool.tile([1, 512], F32, tag="rc")
                    nc.vector.reciprocal(rc[:], cxs[hh][0:1, :])
                    bc = bcast_pool.tile([HD + 1, 512], F32, tag="bc")
                    nc.gpsimd.partition_broadcast(bc[:], rc[:], channels=HD + 1)
                    ct = ctx_pool.tile([HD + 1, 512], BF16, tag="ctx")
                    nc.vector.tensor_mul(ct[:], cxs[hh][:], bc[:])
                    ctx_tiles[(h_local, n)] = ct

            for si in range(4):
                s = 4 * n + si
                for nn in range(2):
                    ps = mm_ps.tile([128, 512], F32, tag="mm")
                    for h_local in range(HPC):
                        nc.tensor.matmul(
                            ps[:],
                            lhsT=ctx_tiles[(h_local, n)][:, ts(si, 128)],
                            rhs=wo_sb[h_local][:, ts(nn, 512)],
                            start=(h_local == 0),
                            stop=(h_local == HPC - 1),
                        )
                    ob = outsb_pool.tile([128, 512], F32, tag="ob")
                    nc.vector.tensor_copy(ob[:], ps[:])
                    nc.sync.dma_start(out[ts(s, 128), ts(nn, 512)], ob[:])

    nc.compile()
    return nc


def _get_nc():
    if "nc" not in _CACHE:
        _CACHE["nc"] = _build_nc()
    return _CACHE["nc"]


def _make_in_maps(inputs):
    x = np.asarray(inputs["x"], np.float32)
    Wq = np.asarray(inputs["Wq"], np.float32)
    Wk = np.asarray(inputs["Wk"], np.float32)
    Wv = np.asarray(inputs["Wv"], np.float32)
    Wo = np.asarray(inputs["Wo"], np.float32)
    bq = np.asarray(inputs["bq"], np.float32)
    bk = np.asarray(inputs["bk"], np.float32)
    bv = np.asarray(inputs["bv"], np.float32)

    in_maps = []
    for core in range(N_CORES):
        b, hg = core // 4, core % 4
        cs = slice(C * hg, C * (hg + 1))
        xT = np.ascontiguousarray(x[b].T).astype(NP_BF16)
        wq_c = np.ascontiguousarray(Wq[:, cs]).astype(NP_BF16)
        wk_c = np.ascontiguousarray(Wk[:, cs]).astype(NP_BF16)
        wv_c = np.ascontiguousarray(Wv[:, cs]).astype(NP_BF16)
        wo_c = np.zeros((HPC, HD + 1, D), np.float32)
        for h in range(HPC):
            r0 = C * hg + HD * h
            wo_c[h, 1 : HD + 1] = Wo[r0 : r0 + HD]
        in_maps.append(
            {
                "xT": xT,
                "wq": wq_c,
                "wk": wk_c,
                "wv": wv_c,
                "wo": wo_c.astype(NP_BF16),
                "bq": np.ascontiguousarray(bq[cs]).reshape(2, 128, 1),
                "bk": np.ascontiguousarray(bk[cs]).reshape(2, 128, 1),
                "bv": np.ascontiguousarray(bv[cs]).reshape(1, C).astype(NP_BF16),
            }
        )
    return in_maps


def run(inputs, trace=False):
    """Run the SPMD kernel; returns (full_output, BassKernelResults)."""
    nc = _get_nc()
    in_maps = _make_in_maps(inputs)
    res = bass_utils.run_bass_kernel_spmd(
        nc, in_maps, core_ids=list(range(N_CORES)), trace=trace
    )
    bo = np.asarray(inputs["bo"], np.float32)
    full = np.empty((2, S, D), np.float32)
    for b in range(2):
        acc = res.results[4 * b]["out"].astype(np.float32).copy()
        for hg in range(1, 4):
            acc += res.results[4 * b + hg]["out"]
        full[b] = acc + bo
    return full, res


def kernel(**inputs):
    full, _ = run(inputs, trace=False)
    return full


# revision 15
# speedup vs baseline: 1.2125x; 1.2125x over previous
"""Multi-head attention (B=2, S=2048, D=1024, H=16) on 8 NeuronCores.

Sharding: core = (batch b, head-group hg) with b in {0,1}, hg in {0..3}.
Each core computes 4 heads (256 of the 1024 hidden dims) for one batch
element and produces a partial output [S, D]; the host sums the 4
head-group partials per batch and adds the output bias.

Per-core dataflow (all matmuls bf16 inputs, fp32 PSUM accumulation):
  Q^T = Wq_c^T @ x^T          [256, S]   (c on partitions)
  K^T = Wk_c^T @ x^T          [256, S]
  V   = x @ Wv_c  (+ bias)    [S, 256]   (s on partitions)
  scores^T[k,q] = K^T(h) slices @ Q^T(h)   (per head, k on partitions,
                                            two heads row-tiled on the PE)
  attn^T = exp(scores^T / 8)  (no max subtraction needed: |scores/8| < ~2)
  ctx_aug^T = [ones | V_h]^T @ attn^T  -> row 0 = softmax denominator,
                                          rows 1..64 = unnormalized ctx^T
  ctx^T = ctx_aug^T * partition_broadcast(1/denom)
  out_partial = sum_h ctx_h^T slices @ Wo_aug_h   (K=65; Wo row 0 is zero
                                                   so the 1.0 row is inert)

Host-side input layouts (pre-tiled so every load is one plain 2D DMA):
  xT  [1024, 2048]  x[b].T                              bf16
  wq/wk/wv [128, 2048]  W[:,cs].reshape(8,128,256) k-tile-major  bf16
  wo  [65, 4096]    per-head [zero_row; Wo_h] side by side       bf16
  bq/bk [128, 2]    bias m-tile columns                          f32
  bv  [1, 256]                                                   bf16
"""

from contextlib import ExitStack

import ml_dtypes
import numpy as np

import concourse.bass as bass
import concourse.mybir as mybir
import concourse.tile as tile
from concourse import bacc
from concourse.bass import ts
from concourse import bass_utils

S = 2048
D = 1024
H = 16
HD = 64
HPC = 4          # heads per core
C = HPC * HD     # 256 hidden dims per core
N_CORES = 8

BF16 = mybir.dt.bfloat16
F32 = mybir.dt.float32
NP_BF16 = ml_dtypes.bfloat16

_CACHE = {}


def _build_nc():
    nc = bacc.Bacc(
        "TRN2", target_bir_lowering=False, debug=False, num_devices=N_CORES
    )

    xT = nc.dram_tensor("xT", [D, S], BF16, kind="ExternalInput").ap()
    wq = nc.dram_tensor("wq", [128, 8 * C], BF16, kind="ExternalInput").ap()
    wk = nc.dram_tensor("wk", [128, 8 * C], BF16, kind="ExternalInput").ap()
    wv = nc.dram_tensor("wv", [128, 8 * C], BF16, kind="ExternalInput").ap()
    wo = nc.dram_tensor("wo", [HD + 1, HPC * D], BF16, kind="ExternalInput").ap()
    bq = nc.dram_tensor("bq", [128, 2], F32, kind="ExternalInput").ap()
    bk = nc.dram_tensor("bk", [128, 2], F32, kind="ExternalInput").ap()
    bv = nc.dram_tensor("bv", [1, C], BF16, kind="ExternalInput").ap()
    out = nc.dram_tensor("out", [S, D], F32, kind="ExternalOutput").ap()

    with tile.TileContext(nc) as tc, ExitStack() as ctx:
        ep = ctx.enter_context

        xt_pool = ep(tc.tile_pool(name="xt", bufs=8))
        w_pool = ep(tc.tile_pool(name="w", bufs=3))
        wo_pool = ep(tc.tile_pool(name="wo", bufs=1))
        small_pool = ep(tc.tile_pool(name="small", bufs=4))
        qk_pool = ep(tc.tile_pool(name="qk", bufs=4))
        vaug_pool = ep(tc.tile_pool(name="vaug", bufs=16))
        ctx_pool = ep(tc.tile_pool(name="ctxp", bufs=16))
        attn_pool = ep(tc.tile_pool(name="attn", bufs=34))
        recip_pool = ep(tc.tile_pool(name="recip", bufs=4))
        bcast_pool = ep(tc.tile_pool(name="bcast", bufs=4))
        outsb_pool = ep(tc.tile_pool(name="outsb", bufs=4))
        mm_ps = ep(tc.tile_pool(name="mmps", bufs=2, space="PSUM"))
        sc_ps = ep(tc.tile_pool(name="scps", bufs=2, space="PSUM"))
        cx_ps = ep(tc.tile_pool(name="cxps", bufs=2, space="PSUM"))

        # ---- loads (weights first; one DMA per tensor, xT in k-tiles) ----
        wk_sb = w_pool.tile([128, 8 * C], BF16, tag="w", name="wk_sb")
        nc.sync.dma_start(wk_sb[:], wk[:])
        wq_sb = w_pool.tile([128, 8 * C], BF16, tag="w", name="wq_sb")
        nc.sync.dma_start(wq_sb[:], wq[:])
        bk_sb = small_pool.tile([128, 2], F32, tag="bqk", name="bk_sb")
        nc.sync.dma_start(bk_sb[:], bk[:])
        bq_sb = small_pool.tile([128, 2], F32, tag="bqk", name="bq_sb")
        nc.sync.dma_start(bq_sb[:], bq[:])

        xt = []
        for k in range(8):
            t = xt_pool.tile([128, S], BF16, tag="xt", name=f"xt_{k}")
            nc.sync.dma_start(t[:], xT[ts(k, 128), :])
            xt.append(t)

        wv_sb = w_pool.tile([128, 8 * C], BF16, tag="w", name="wv_sb")
        nc.sync.dma_start(wv_sb[:], wv[:])
        bv_row = small_pool.tile([1, C], BF16, tag="bv")
        nc.sync.dma_start(bv_row[:], bv[:])
        ones_row = small_pool.tile([1, 128], BF16, tag="ones")
        nc.vector.memset(ones_row[:], 1.0)
        wo_sb = wo_pool.tile([HD + 1, HPC * D], BF16, tag="wo", name="wo_sb")
        nc.sync.dma_start(wo_sb[:], wo[:])

        # ---- projection emitters ----
        kt = [None, None]
        qt = [None, None]

        def emit_kq_round(dst_list, w_t, b_t, m, n, label):
            if dst_list[m] is None:
                dst_list[m] = qk_pool.tile(
                    [128, S], BF16, tag="qk", name=f"{label}_{m}"
                )
            dst = dst_list[m]
            ps = mm_ps.tile([128, 512], F32, tag="mm", name=f"ps{label}_{m}_{n}")
            for k in range(8):
                nc.tensor.matmul(
                    ps[:],
                    lhsT=w_t[:, 256 * k + 128 * m : 256 * k + 128 * m + 128],
                    rhs=xt[k][:, ts(n, 512)],
                    start=(k == 0),
                    stop=(k == 7),
                )
            nc.vector.tensor_scalar(
                dst[:, ts(n, 512)],
                ps[:],
                b_t[:, m : m + 1],
                None,
                mybir.AluOpType.add,
            )

        vaug = []

        def emit_v():
            for s in range(16):
                ps = mm_ps.tile([128, C], F32, tag="mm", name=f"psv_{s}")
                for k in range(8):
                    nc.tensor.matmul(
                        ps[:],
                        lhsT=xt[k][:, ts(s, 128)],
                        rhs=wv_sb[:, ts(k, C)],
                        start=(k == 0),
                        stop=False,
                    )
                nc.tensor.matmul(
                    ps[:],
                    lhsT=ones_row[:],
                    rhs=bv_row[:],
                    start=False,
                    stop=True,
                )
                vt = vaug_pool.tile(
                    [128, HPC * (HD + 1)], BF16, tag="vaug", name=f"vaug_{s}"
                )
                vt3 = vt[:].rearrange("p (h x) -> p h x", x=HD + 1)
                nc.vector.memset(vt3[:, :, 0:1], 1.0)
                nc.vector.tensor_copy(
                    vt3[:, :, 1 : HD + 1],
                    ps[:].rearrange("p (h d) -> p h d", d=HD),
                )
                vaug.append(vt)

        # ---- attention unit: one (q-chunk, head-pair) ----
        # Split into a scores/exp part and a ctx part so the first unit's
        # scores can be traced before the V projection (ACT ramps early)
        # while its ctx matmuls come after vaug exists.
        ctx_tiles = {}

        def emit_scores(n, p, j):
            scs = [
                sc_ps.tile(
                    [128, 1024], F32, tag="sc", name=f"sc_{n}_{p}_{j}_{hh}"
                )
                for hh in range(2)
            ]
            for tt in range(2):
                t = 2 * j + tt
                for hh in range(2):
                    nc.tensor.matmul(
                        scs[hh][:, ts(tt, 512)],
                        lhsT=kt[p][64 * hh : 64 * hh + 64, ts(t, 128)],
                        rhs=qt[p][64 * hh : 64 * hh + 64, ts(n, 512)],
                        start=True,
                        stop=True,
                        tile_position=(64 * hh, 0),
                    )
            ats = []
            for hh in range(2):
                at = attn_pool.tile(
                    [128, 1024], BF16, tag="at", name=f"at_{n}_{p}_{j}_{hh}"
                )
                nc.scalar.activation(
                    at[:],
                    scs[hh][:],
                    mybir.ActivationFunctionType.Exp,
                    scale=0.125,
                )
                ats.append(at)
            return ats

        def emit_ctx_mm(n, p, j, ats, cxs):
            for tt in range(2):
                t = 2 * j + tt
                for hh in range(2):
                    h_local = 2 * p + hh
                    nc.tensor.matmul(
                        cxs[hh][:],
                        lhsT=vaug[t][:, 65 * h_local : 65 * h_local + 65],
                        rhs=ats[hh][:, ts(tt, 512)],
                        start=(t == 0),
                        stop=(t == 15),
                    )

        def emit_norm(n, p, cxs):
            for hh in range(2):
                h_local = 2 * p + hh
                rc = recip_pool.tile([1, 512], F32, tag="rc", name=f"rc_{n}_{p}_{hh}")
                nc.vector.reciprocal(rc[:], cxs[hh][0:1, :])
                bc = bcast_pool.tile(
                    [HD + 1, 512], F32, tag="bc", name=f"bc_{n}_{p}_{hh}"
                )
                nc.gpsimd.partition_broadcast(bc[:], rc[:], channels=HD + 1)
                ct = ctx_pool.tile(
                    [HD + 1, 512], BF16, tag="ctx", name=f"ctx_{n}_{p}_{hh}"
                )
                nc.vector.tensor_mul(ct[:], cxs[hh][:], bc[:])
                ctx_tiles[(h_local, n)] = ct

        def emit_attention(n, p):
            cxs = [
                cx_ps.tile([HD + 1, 512], F32, tag="cx", name=f"cx_{n}_{p}_{hh}")
                for hh in range(2)
            ]
            for j in range(8):
                ats = emit_scores(n, p, j)
                emit_ctx_mm(n, p, j, ats, cxs)
            emit_norm(n, p, cxs)

        def emit_outproj(n, si_range=range(4)):
            for si in si_range:
                s = 4 * n + si
                for nn in range(2):
                    ps = mm_ps.tile(
                        [128, 512], F32, tag="mm", name=f"pso_{s}_{nn}"
                    )
                    for h_local in range(HPC):
                        nc.tensor.matmul(
                            ps[:],
                            lhsT=ctx_tiles[(h_local, n)][:, ts(si, 128)],
                            rhs=wo_sb[
                                :, 1024 * h_local + 512 * nn : 1024 * h_local + 512 * nn + 512
                            ],
                            start=(h_local == 0),
                            stop=(h_local == HPC - 1),
                        )
                    ob = outsb_pool.tile(
                        [128, 512], F32, tag="ob", name=f"ob_{s}_{nn}"
                    )
                    nc.vector.tensor_copy(ob[:], ps[:])
                    nc.sync.dma_start(out[ts(s, 128), ts(nn, 512)], ob[:])

        # ---- emission order ----
        # Interleave K/Q chunk-0 rounds with the first two units' scores so
        # ACT (near-critical engine) ramps as early as possible; V emitted
        # after as PE backfill; ctx parts follow V; outproj is deferred one
        # chunk so it backfills PE stalls during ACT-bound stretches.
        emit_kq_round(kt, wk_sb, bk_sb, 0, 0, "k")
        emit_kq_round(qt, wq_sb, bq_sb, 0, 0, "q")
        ats00, ats10 = [], []
        for j in range(2):
            ats00.append(emit_scores(0, 0, j))
        emit_kq_round(kt, wk_sb, bk_sb, 0, 1, "k")
        for j in range(2, 4):
            ats00.append(emit_scores(0, 0, j))
        emit_kq_round(kt, wk_sb, bk_sb, 0, 2, "k")
        for j in range(4, 6):
            ats00.append(emit_scores(0, 0, j))
        emit_kq_round(kt, wk_sb, bk_sb, 0, 3, "k")
        for j in range(6, 8):
            ats00.append(emit_scores(0, 0, j))
        emit_kq_round(qt, wq_sb, bq_sb, 0, 1, "q")
        emit_kq_round(qt, wq_sb, bq_sb, 0, 2, "q")
        emit_kq_round(qt, wq_sb, bq_sb, 0, 3, "q")
        emit_kq_round(kt, wk_sb, bk_sb, 1, 0, "k")
        emit_kq_round(qt, wq_sb, bq_sb, 1, 0, "q")
        ats10.append(emit_scores(0, 1, 0))
        ats10.append(emit_scores(0, 1, 1))
        emit_kq_round(kt, wk_sb, bk_sb, 1, 1, "k")
        ats10.append(emit_scores(0, 1, 2))
        ats10.append(emit_scores(0, 1, 3))
        emit_kq_round(kt, wk_sb, bk_sb, 1, 2, "k")
        ats10.append(emit_scores(0, 1, 4))
        ats10.append(emit_scores(0, 1, 5))
        emit_kq_round(kt, wk_sb, bk_sb, 1, 3, "k")
        ats10.append(emit_scores(0, 1, 6))
        ats10.append(emit_scores(0, 1, 7))
        emit_kq_round(qt, wq_sb, bq_sb, 1, 1, "q")
        emit_kq_round(qt, wq_sb, bq_sb, 1, 2, "q")
        emit_kq_round(qt, wq_sb, bq_sb, 1, 3, "q")
        emit_v()
        for p, ats in ((0, ats00), (1, ats10)):
            cxs = [
                cx_ps.tile([HD + 1, 512], F32, tag="cx", name=f"cx_0_{p}_{hh}")
                for hh in range(2)
            ]
            for j in range(8):
                emit_ctx_mm(0, p, j, ats[j], cxs)
            emit_norm(0, p, cxs)
        emit_attention(1, 0)
        emit_attention(1, 1)
        emit_outproj(0)
        emit_attention(2, 0)
        emit_attention(2, 1)
        emit_outproj(1)
        emit_attention(3, 0)
        emit_attention(3, 1)
        emit_outproj(2)
        emit_outproj(3)

    nc.compile()
    return nc


def _get_nc():
    if "nc" not in _CACHE:
        _CACHE["nc"] = _build_nc()
    return _CACHE["nc"]


def _make_in_maps(inputs):
    x = np.asarray(inputs["x"], np.float32)
    Wq = np.asarray(inputs["Wq"], np.float32)
    Wk = np.asarray(inputs["Wk"], np.float32)
    Wv = np.asarray(inputs["Wv"], np.float32)
    Wo = np.asarray(inputs["Wo"], np.float32)
    bq = np.asarray(inputs["bq"], np.float32)
    bk = np.asarray(inputs["bk"], np.float32)
    bv = np.asarray(inputs["bv"], np.float32)

    def tile_w(w_slice):
        # [1024, 256] -> [128, 8*256] with k-tile-major free dim
        return np.ascontiguousarray(
            w_slice.reshape(8, 128, C).transpose(1, 0, 2).reshape(128, 8 * C)
        ).astype(NP_BF16)

    in_maps = []
    for core in range(N_CORES):
        b, hg = core // 4, core % 4
        cs = slice(C * hg, C * (hg + 1))
        xT = np.ascontiguousarray(x[b].T).astype(NP_BF16)
        wo_c = np.zeros((HD + 1, HPC * D), np.float32)
        for h in range(HPC):
            r0 = C * hg + HD * h
            wo_c[1 : HD + 1, D * h : D * (h + 1)] = Wo[r0 : r0 + HD]
        in_maps.append(
            {
                "xT": xT,
                "wq": tile_w(Wq[:, cs]),
                "wk": tile_w(Wk[:, cs]),
                "wv": tile_w(Wv[:, cs]),
                "wo": wo_c.astype(NP_BF16),
                "bq": np.ascontiguousarray(bq[cs].reshape(2, 128).T),
                "bk": np.ascontiguousarray(bk[cs].reshape(2, 128).T),
                "bv": np.ascontiguousarray(bv[cs].reshape(1, C)).astype(NP_BF16),
            }
        )
    return in_maps


def run(inputs, trace=False):
    """Run the SPMD kernel; returns (full_output, BassKernelResults)."""
    nc = _get_nc()
    in_maps = _make_in_maps(inputs)
    res = bass_utils.run_bass_kernel_spmd(
        nc, in_maps, core_ids=list(range(N_CORES)), trace=trace
    )
    bo = np.asarray(inputs["bo"], np.float32)
    full = np.empty((2, S, D), np.float32)
    for b in range(2):
        acc = res.results[4 * b]["out"].astype(np.float32).copy()
        for hg in range(1, 4):
            acc += res.results[4 * b + hg]["out"]
        full[b] = acc + bo
    return full, res


def kernel(**inputs):
    full, _ = run(inputs, trace=False)
    return full
